# revision 5
# baseline (speedup 1.0000x reference)
"""Bidirectional ConvLSTM block for Trainium2 (Bass/Tile), 8-core SPMD.

Problem: x [S=16, B=4, Cin=32, H=128, W=128] f32, Wf/Wb [128, 64, 3, 3],
bf/bb [128].  Output [S, B, 2*Co=64, H, W]: forward ConvLSTM hidden states
concat backward ConvLSTM (run on time-reversed x, not re-flipped).

Sharding: 8 independent recurrences = 2 directions x 4 batch elements.
Core k runs direction d=k//4 on batch b=k%4.  No cross-core communication.

v2 design (vs v1 baseline):
  - 5 matmul passes per gate instead of 6, via two act tiles with
    physically-shifted copies (planes are 128-wide CONTIGUOUS, 132 rows:
    row p holds image row p-1; no column padding - shifted copies wrap at
    row edges and the wrap column is memset to zero):
      A tile: [x>>1 | x | x<<1 | h]        (x at col-shifts +1,0,-1; h center)
      B tile: [h>>1 | h<<1 | h(up)>>1 | h(up)<<1]
    Passes per gate: A at row offsets -1,0,+1 (each covers 3 x-taps + the
    middle-column h-tap), B at row offsets -1,+1 (covering the 6 h-taps in
    columns +-1; 2 blocks of the +1 pass are zero-weighted).
    18 of 20 K-blocks carry real taps (vs 18/24 in v1) -> 10240 vs 12288
    matmuls.
  - h write-back is CONTIGUOUS: pointwise writes h (bf16) into the A-tile
    center plane as 4x [32,512] 1KB-run DMAs per group; the 4 shifted
    copies in the B tile are big contiguous slab copies (one per 32-row
    pair) from the A center plane, +-1/+-129 element offsets.  This cuts
    DMA descriptors/step from ~9300 tiny (256B) to ~1700 mostly-large,
    eliminating the DMA-backlog stalls that idled the PE 220us in v1.
  - y output: one [32, 8KB-per-partition] bf16 DMA per 32-row pair straight
    from the A center plane (h already lives there); host casts to f32.
  - matmul phases keyed by (gate, pass): one weight block feeds 8 matmuls
    (2 groups x 4 col-tiled spatial strips), so LDWEIGHTS overlaps rhs
    streaming of the previous strip instead of serializing per-matmul.
"""

import os
import sys

import numpy as np

for _p in ("/opt/trn_rl_repo", "/root/.axon_site/_ro/trn_rl_repo"):
    if os.path.isdir(_p) and _p not in sys.path:
        sys.path.insert(0, _p)

import ml_dtypes  # noqa: E402
import concourse.bass as bass  # noqa: E402,F401
import concourse.mybir as mybir  # noqa: E402
from concourse import bacc, tile  # noqa: E402
from concourse.ap import AP as BassAP  # noqa: E402
from concourse.bass_utils import run_bass_kernel_spmd  # noqa: E402

F32 = mybir.dt.float32
BF16 = mybir.dt.bfloat16
AF = mybir.ActivationFunctionType

S, B, CIN, H, W = 16, 4, 32, 128, 128
CO = 32
NR = 132                       # plane rows: 0 pad, 1..128 image, 129.. pad
PL = NR * W                    # 16896 elems per plane (contiguous, 128-wide)
NSP = H * W                    # 16384
NT = 512                       # spatial positions per matmul tile (4 rows)
TPG = 4                        # col-tiled strips (spatial tiles per group)
GROUPS = 8                     # 16 image rows per group
N_CORES = 8

# B-tile block shifts (sy, sx): plane(p,c) = h(p-1-sy, c-sx)
# v7: only the two column-shifted blocks b0/b1; the (0,+-1) taps that v2-v4
# got from the row-shifted duplicates b2/b3 come from a third B pass at
# ry=0 instead.  Costs one extra (half-useful) matmul pass per gate
# (PE stream 34.7 -> 41 us/step, still under the DMA ceiling) but removes
# half the B-slab write-back: DMA drops 9.4 -> 7.3 MB/step, below the
# ~220 GB/s fabric's capacity within a step, so the PE stops stalling
# on the write-back chain.
B_SHIFTS = [(0, 1), (0, -1)]
# pass list per gate: (which tile, row offset)
PASSES = [("A", -1), ("A", 0), ("A", 1), ("B", -1), ("B", 0), ("B", 1)]


def build_kernel(nc, tc, x_ap, w_ap, b_ap, y_ap, n_steps):
    ctx_pools = []

    def pool(**kw):
        p = tc.tile_pool(**kw)
        ctx_pools.append(p)
        return p.__enter__()

    const = pool(name="const", bufs=1)
    tmp = pool(name="tmp", bufs=2)
    psum = pool(name="psum", bufs=8, space="PSUM")

    # Persistent tiles
    A = [const.tile([128, PL], BF16, tag=f"actA{i}", name=f"actA{i}") for i in range(2)]
    Bt = [const.tile([128, PL], BF16, tag=f"actB{i}", name=f"actB{i}") for i in range(2)]
    ctile = const.tile([128, GROUPS * NT], F32, tag="c")
    wsb = const.tile([128, 24 * 32], BF16, tag="w")
    bsb = const.tile([128, 4], F32, tag="bias")

    # head loads spread across rings so they overlap
    nc.scalar.dma_start(wsb[:, :], w_ap)
    nc.scalar.dma_start(bsb[:, :], b_ap)

    # one-time zero init.  B0 and A0's h block are fully read at t=0 and
    # must be all-zero; later steps only need the pad rows (never written by
    # the per-step h write-back) to be zero.  Chunked 34-row-wise so the
    # rows pair-0's first matmuls read are zeroed (and copied into A0)
    # first - cuts the t=0 head stall.
    # zero chunk 0 of B0 once, then source ALL A0-zero chunks from it so the
    # A0 fills don't serialize behind the remaining B0 memsets
    nc.vector.memset(Bt[0][:, 0 : 33 * W], 0.0)
    for ch in range(4):
        c0, c1 = ch * 33 * W, min((ch + 1) * 33 * W, PL)
        nc.gpsimd.dma_start(A[0][96:128, c0:c1], Bt[0][0:32, 0 : c1 - c0])
    for ch in range(1, 4):
        c0, c1 = ch * 33 * W, min((ch + 1) * 33 * W, PL)
        nc.vector.memset(Bt[0][:, c0:c1], 0.0)
    nc.vector.memset(ctile[:, :], 0.0)
    nc.vector.memset(Bt[1][:, 128 * W :], 0.0)          # pad rows 128..131
    nc.vector.memset(Bt[1][:, 0:W], 0.0)                # pad row 0
    nc.vector.memset(A[1][96:128, 0:W], 0.0)            # pad row 0
    nc.vector.memset(A[1][96:128, 129 * W :], 0.0)      # pad rows 129..131
    # b2/b3 partitions are never written in v7 but still streamed (x0
    # weights); they must hold zeros, not junk/NaN bit patterns
    nc.gpsimd.memset(Bt[1][64:128, 0 : 66 * W], 0.0)
    nc.gpsimd.memset(Bt[1][64:128, 66 * W :], 0.0)

    XCH = PL // 4  # 4224 elems -> 8.4KB descriptors, fine-grained ring interleave

    def load_x(t, chunk=None):
        # host provides 3 pre-shifted padded planes packed on partitions 0-95.
        # Chunked along the free dim so no descriptor exceeds ~8KB: SDMA
        # engines switch logical queues at packet granularity, and 33KB
        # descriptors block the latency-critical h write-backs for ~3us each.
        # Rides the sync/HWDGE ring: with the merged center writes (16
        # issues/step) the sync sequencer has slack, and keeping x off the
        # SWDGE ring keeps the slab-copy latency chain clean.
        chunks = range(4) if chunk is None else [chunk]
        for ch in chunks:
            nc.sync.dma_start(
                A[t % 2][0:96, XCH * ch : XCH * (ch + 1)],
                x_ap[t, :, XCH * ch : XCH * (ch + 1)],
            )

    load_x(0)

    deferred = []
    for t in range(n_steps):
        A_cur, B_cur = A[t % 2], Bt[t % 2]
        A_nxt, B_nxt = A[(t + 1) % 2], Bt[(t + 1) % 2]

        def pointwise(grp, zb, hbf, half):
            csl = ctile[:, grp * NT : (grp + 1) * NT]
            si = tmp.tile([128, NT], F32, tag="si", name=f"si{t}_{grp}")
            sf = tmp.tile([128, NT], F32, tag="sf", name=f"sf{t}_{grp}")
            so = tmp.tile([128, NT], F32, tag="so", name=f"so{t}_{grp}")
            tg = tmp.tile([128, NT], F32, tag="tg", name=f"tg{t}_{grp}")
            nc.scalar.activation(si[:, :], zb[0][:, :], AF.Sigmoid, bias=bsb[:, 0:1])
            nc.scalar.activation(sf[:, :], zb[1][:, :], AF.Sigmoid, bias=bsb[:, 1:2])
            nc.scalar.activation(so[:, :], zb[2][:, :], AF.Sigmoid, bias=bsb[:, 2:3])
            nc.scalar.activation(tg[:, :], zb[3][:, :], AF.Tanh, bias=bsb[:, 3:4])

            t2 = tmp.tile([128, NT], F32, tag="t2", name=f"t2_{t}_{grp}")
            t3 = tmp.tile([128, NT], F32, tag="t3", name=f"t3_{t}_{grp}")
            # t3 only needs sf (2nd activation) - run before t2 so the DVE
            # isn't blocked behind tanh_g on the critical chain
            nc.vector.tensor_mul(t3[:, :], sf[:, :], csl)
            nc.vector.tensor_mul(t2[:, :], si[:, :], tg[:, :])
            nc.vector.tensor_add(csl, t2[:, :], t3[:, :])

            tcn = tmp.tile([128, NT], F32, tag="tcn", name=f"tcn{t}_{grp}")
            nc.scalar.activation(tcn[:, :], csl, AF.Tanh)
            nc.vector.tensor_mul(
                hbf[:, half * NT : (half + 1) * NT], so[:, :], tcn[:, :]
            )

        def emit_centers(pair, hbf, A_nxt):
            # h -> A-center plane.  Both groups of the pair are merged into
            # one DMA per strip j ([32p, 2 segs of 1KB at 16-row stride],
            # 3-dim APs): 16 sync issues/step instead of 32 - the
            # ~0.65us/issue HWDGE sequencer cost was the v2 bottleneck
            # (sync engine 76% occupied).
            for j in range(TPG):
                d0 = (32 * pair + 4 * j + 1) * W
                v = A_nxt[96:128, d0 : d0 + NT]
                dst = BassAP(v.tensor, v.offset, [list(v.ap[0]), [16 * W, 2], [1, NT]])
                src = hbf[32 * j : 32 * j + 32, :].rearrange(
                    "p (g n) -> p g n", g=2
                )
                nc.sync.dma_start(dst, src)

        deferred_prev = deferred[:]
        del deferred[:]

        for pair in range(4):
            grps = (2 * pair, 2 * pair + 1)
            zbs = [
                [
                    psum.tile([128, NT], F32, tag="z", name=f"z{t}_{grp}_{g}")
                    for g in range(4)
                ]
                for grp in grps
            ]
            for g in range(4):
                for pi, (tsel, ry) in enumerate(PASSES):
                    src = A_cur if tsel == "A" else B_cur
                    col = (g * 6 + pi) * 32
                    lhsT = wsb[:, col : col + 32]
                    for j in range(TPG):
                        for gi, grp in enumerate(grps):
                            r0 = (16 * grp + 4 * j + 1 + ry) * W
                            nc.tensor.matmul(
                                zbs[gi][g][32 * j : 32 * j + 32, :],
                                lhsT,
                                src[:, r0 : r0 + NT],
                                start=(pi == 0),
                                stop=(pi == 5),
                                skip_group_check=True,
                                tile_position=(0, 32 * j),
                            )
            if pair == 1:
                # deferred end-of-previous-step emissions: after PAIR-1's
                # matmul emissions so neither pair-0 nor pair-1 matmuls
                # sem-lane-sweep them.  Legal: the deferred writes touch
                # A/B rows >= 96 and x chunk 3 (plane rows 99+), while
                # pair-0/1 matmuls read rows <= 66; pair-2 (rows 64..98)
                # is the first true reader of rows 96-98.
                for fn in deferred_prev:
                    fn()
                del deferred_prev[:]

            hbf = tmp.tile([128, 2 * NT], BF16, tag="hbf", name=f"hbf{t}_{pair}")
            if t + 1 < n_steps:
                pointwise(grps[0], zbs[0], hbf, 0)
                pointwise(grps[1], zbs[1], hbf, 1)
                if pair == 3:
                    # defer the last pair's centers into the next step so its
                    # first matmuls don't sweep them up in the sync sem-lane
                    # wait
                    deferred.append(
                        lambda pair=pair, hbf=hbf, A_nxt=A_nxt: emit_centers(
                            pair, hbf, A_nxt
                        )
                    )
                else:
                    emit_centers(pair, hbf, A_nxt)
            else:
                # last step: y straight from hbf (2-seg merged DMAs, no
                # A-center hop) - nothing else reads h after the final step
                pointwise(grps[0], zbs[0], hbf, 0)
                pointwise(grps[1], zbs[1], hbf, 1)
                for j in range(TPG):
                    d0 = (32 * pair + 4 * j) * W
                    v = y_ap[t][:, d0 : d0 + NT]
                    dst = BassAP(
                        v.tensor, v.offset, [list(v.ap[0]), [16 * W, 2], [1, NT]]
                    )
                    src = hbf[32 * j : 32 * j + 32, :].rearrange(
                        "p (g n) -> p g n", g=2
                    )
                    nc.sync.dma_start(dst, src)

            # shifted h copies into B_nxt for this 32-row slab + wrap-col
            # cleanup.  These ride the gpsimd/SWDGE path: a DMA issue that
            # must *wait* on a completion sem (slab after center) blocks its
            # sequencer, and gpsimd has nothing latency-critical behind it.
            lo = 32 * pair

            def emit_writeback(lo=lo, t=t, A_nxt=A_nxt, B_nxt=B_nxt, pair=pair):
                if t + 1 < n_steps:
                    for b_, (sy, sx) in enumerate(B_SHIFTS):
                        r_lo, r_hi = lo + 1 + sy, lo + 32 + sy  # dst rows
                        d0, d1 = r_lo * W, (r_hi + 1) * W
                        delta = -(sy * W + sx)
                        # trim one wrap-col element at the end (sx=-1) or
                        # start (sx=+1): src read stays within the pair rows
                        if sx == -1:
                            dd0, dd1 = d0, d1 - 1
                        else:
                            dd0, dd1 = d0 + 1, d1
                        nc.gpsimd.dma_start(
                            B_nxt[32 * b_ : 32 * b_ + 32, dd0:dd1],
                            A_nxt[96:128, dd0 + delta : dd1 + delta],
                        )
                    for b_, (sy, sx) in enumerate(B_SHIFTS):
                        r_lo, r_hi = lo + 1 + sy, lo + 32 + sy
                        wc = 0 if sx == 1 else W - 1
                        br = B_nxt[32 * b_ : 32 * b_ + 32, :].rearrange(
                            "p (r w) -> p r w", w=W
                        )
                        nc.gpsimd.memset(br[:, r_lo : r_hi + 1, wc : wc + 1], 0.0)
                    load_x(t + 1, chunk=pair)

            if pair == 3:
                # defer the end-of-step write-back emission into the next
                # step (after its pair-0 matmuls): nothing in the next step's
                # first pair reads these ranges, so emitting them later keeps
                # the pair-0 matmuls from waiting on this tail work.
                deferred.append(emit_writeback)
            else:
                emit_writeback()

        if t + 1 < n_steps:
            def emit_y(t=t, A_nxt=A_nxt):
                nc.sync.dma_start(y_ap[t], A_nxt[96:128, W : W + NSP])

            deferred.append(emit_y)

    for fn in deferred:
        fn()
    del deferred[:]

    for p in reversed(ctx_pools):
        p.__exit__(None, None, None)


def build_program(n_steps=S):
    nc = bacc.Bacc(
        "TRN2",
        target_bir_lowering=False,
        debug=False,
        enable_asserts=False,
        num_devices=N_CORES,
    )
    x_d = nc.dram_tensor("x", [n_steps, 96, PL], BF16, kind="ExternalInput")
    w_d = nc.dram_tensor("w", [128, 24 * 32], BF16, kind="ExternalInput")
    b_d = nc.dram_tensor("bias", [128, 4], F32, kind="ExternalInput")
    y_d = nc.dram_tensor("y", [n_steps, CO, NSP], BF16, kind="ExternalOutput")
    with tile.TileContext(nc) as tc:
        build_kernel(nc, tc, x_d.ap(), w_d.ap(), b_d.ap(), y_d.ap(), n_steps)
    nc.compile()
    return nc


def pack_weights(Wd):
    """Wd [128, 64, 3, 3] f32 -> lhsT blocks [128, 24*32] bf16.

    Block b = gate*6 + pass. K-row blocks of 32 channels; col m = out ch.
    A passes (0-2, ry=p-1): [x tap (ry,-1) | x (ry,0) | x (ry,+1) | h (ry,0)]
    B passes (3-5, ry=p-4): h taps [(ry,-1) | (ry,+1) | 0 | 0]
    """
    wp = np.zeros((128, 24, 32), np.float32)
    for g in range(4):
        Wg = Wd[g * 32 : (g + 1) * 32]  # [32(m), 64, 3, 3]
        for p in range(3):
            blk = wp[:, g * 6 + p, :]
            blk[0:32, :] = Wg[:, 0:32, p, 0].T
            blk[32:64, :] = Wg[:, 0:32, p, 1].T
            blk[64:96, :] = Wg[:, 0:32, p, 2].T
            blk[96:128, :] = Wg[:, 32:64, p, 1].T
        for p in range(3):
            blk = wp[:, g * 6 + 3 + p, :]
            blk[0:32, :] = Wg[:, 32:64, p, 0].T
            blk[32:64, :] = Wg[:, 32:64, p, 2].T
    return wp.reshape(128, 24 * 32).astype(ml_dtypes.bfloat16) if not globals().get("_NO_QUANT") else wp.reshape(128, 24 * 32)


def pack_bias(bd):
    """bd [128] f32 -> [128, 4] f32 (partition p = 32*tile + ch)."""
    bp = np.zeros((128, 4), np.float32)
    for g in range(4):
        bp[:, g] = np.tile(bd[g * 32 : (g + 1) * 32], 4)
    return bp


def pack_x(xc):
    """xc [S, 32, 128, 128] f32 -> [S, 96, NR*W] bf16: 3 shifted planes.

    partitions 0-31: x>>1 (plane(p,c)=x(p-1,c-1)), 32-63: center,
    64-95: x<<1 (plane(p,c)=x(p-1,c+1)). Host-exact zero padding.
    """
    n = xc.shape[0]
    xp = np.zeros((n, 96, NR, W), np.float32)
    xp[:, 32:64, 1 : H + 1, :] = xc
    xp[:, 0:32, 1 : H + 1, 1:] = xc[:, :, :, : W - 1]
    xp[:, 64:96, 1 : H + 1, : W - 1] = xc[:, :, :, 1:]
    return xp.reshape(n, 96, PL).astype(ml_dtypes.bfloat16) if not globals().get("_NO_QUANT") else xp.reshape(n, 96, PL)


def make_in_maps(x, Wf, bf, Wb, bb, n_steps=S):
    wpacks = [pack_weights(np.asarray(Wf, np.float32)),
              pack_weights(np.asarray(Wb, np.float32))]
    bpacks = [pack_bias(np.asarray(bf, np.float32)),
              pack_bias(np.asarray(bb, np.float32))]
    x = np.asarray(x, np.float32)
    in_maps = []
    for k in range(N_CORES):
        d, b = k // 4, k % 4
        xc = x[:n_steps, b] if d == 0 else x[::-1][:n_steps, b]
        in_maps.append(
            {
                "x": pack_x(xc),
                "w": wpacks[d],
                "bias": bpacks[d],
            }
        )
    return in_maps


_CACHED_NC = None


def kernel(x, Wf, bf, Wb, bb):
    global _CACHED_NC
    if _CACHED_NC is None:
        _CACHED_NC = build_program(S)
    nc = _CACHED_NC
    in_maps = make_in_maps(x, Wf, bf, Wb, bb)
    res = run_bass_kernel_spmd(nc, in_maps, core_ids=list(range(N_CORES)))
    out = np.empty((S, B, 2 * CO, H, W), np.float32)
    for k in range(N_CORES):
        d, b = k // 4, k % 4
        yk = res.results[k]["y"].astype(np.float32).reshape(S, CO, H, W)
        out[:, b, d * CO : (d + 1) * CO] = yk
    return out


if __name__ == "__main__":
    import jax

    jax.config.update("jax_platforms", "cpu")
    rng = np.random.default_rng(0)
    x = rng.standard_normal((S, B, CIN, H, W), np.float32)
    Wf = (rng.standard_normal((128, 64, 3, 3)) * 0.05).astype(np.float32)
    Wb = (rng.standard_normal((128, 64, 3, 3)) * 0.05).astype(np.float32)
    bf = np.zeros(128, np.float32)
    bb = np.zeros(128, np.float32)
    y = kernel(x, Wf, bf, Wb, bb)
    print("out", y.shape, y.dtype)



# revision 6
# speedup vs baseline: 1.0071x; 1.0071x over previous
"""Bidirectional ConvLSTM block for Trainium2 (Bass/Tile), 8-core SPMD.

Problem: x [S=16, B=4, Cin=32, H=128, W=128] f32, Wf/Wb [128, 64, 3, 3],
bf/bb [128].  Output [S, B, 2*Co=64, H, W]: forward ConvLSTM hidden states
concat backward ConvLSTM (run on time-reversed x, not re-flipped).

Sharding: 8 independent recurrences = 2 directions x 4 batch elements.
Core k runs direction d=k//4 on batch b=k%4.  No cross-core communication.

v10 design (evolution of the v2 5-pass layout; 1162 -> 707 us):
  - Act tiles, 128-wide contiguous planes of 132 rows (row p = image row
    p-1), with physically-shifted copies:
      A tile: [x>>1 | x | x<<1 | h]   (x at col-shifts +1,0,-1; h center)
      B tile: [h>>1 | h<<1 | 0 | 0]
    6 passes/gate: A at row offsets -1,0,+1 (3 x-taps + center h-tap each),
    B at row offsets -1,0,+1 (the 6 column-shifted h-taps; upper 64 K
    partitions zero-weighted).  v2-v4 used 5 passes with 2 extra
    row-shifted B blocks; dropping them costs ~6 us/step of PE stream but
    halves the B write-back (DMA 9.4 -> 7.3 MB/step), un-saturating the
    ~220 GB/s SDMA fabric: the PE had stalled ~2x/step on the write-back
    chain and the stalls re-throttled the HAM clock gate (25% of v2 ran
    at 1.2 GHz).  v10 runs ~93% PE-busy, HAM warm throughout.
  - h write-back: one [32p, 2x1KB-seg] 3-dim-AP DMA per strip covering
    both groups of a 32-row pair (16 sync issues/step; the HWDGE
    sequencer costs ~0.65-1.0 us per dma_start instruction and was 76%
    occupied in v2 with 32+ issues).  B-tile blocks are 2 contiguous slab
    copies per pair (gpsimd/SWDGE) + Pool wrap-column memsets.
  - End-of-step emissions (pair-3 centers/slabs/y/x-prefetch) deferred
    past the NEXT step's pair-1 matmul emissions: Tile's monotonic DMA
    sem-lane thresholds would otherwise make pair-0/1 matmuls wait on
    them (they only touch rows >= 96 / x chunk 3; pair 2 is the first
    true reader).  NOTE: head DMA emission order/ring assignment shifts
    the global sem-lane rotation - v9's x-split to the scalar ring cost
    +9 us/step of per-instruction wait overhead.  Change with care.
  - Last step: y written straight from the hbf tiles (2-seg merged DMAs,
    no A-center hop) to shorten the drain tail.
"""

import os
import sys

import numpy as np

for _p in ("/opt/trn_rl_repo", "/root/.axon_site/_ro/trn_rl_repo"):
    if os.path.isdir(_p) and _p not in sys.path:
        sys.path.insert(0, _p)

import ml_dtypes  # noqa: E402
import concourse.bass as bass  # noqa: E402,F401
import concourse.mybir as mybir  # noqa: E402
from concourse import bacc, tile  # noqa: E402
from concourse.ap import AP as BassAP  # noqa: E402
from concourse.bass_utils import run_bass_kernel_spmd  # noqa: E402

F32 = mybir.dt.float32
BF16 = mybir.dt.bfloat16
AF = mybir.ActivationFunctionType

S, B, CIN, H, W = 16, 4, 32, 128, 128
CO = 32
NR = 132                       # plane rows: 0 pad, 1..128 image, 129.. pad
PL = NR * W                    # 16896 elems per plane (contiguous, 128-wide)
NSP = H * W                    # 16384
NT = 512                       # spatial positions per matmul tile (4 rows)
TPG = 4                        # col-tiled strips (spatial tiles per group)
GROUPS = 8                     # 16 image rows per group
N_CORES = 8

# B-tile block shifts (sy, sx): plane(p,c) = h(p-1-sy, c-sx)
# v7: only the two column-shifted blocks b0/b1; the (0,+-1) taps that v2-v4
# got from the row-shifted duplicates b2/b3 come from a third B pass at
# ry=0 instead.  Costs one extra (half-useful) matmul pass per gate
# (PE stream 34.7 -> 41 us/step, still under the DMA ceiling) but removes
# half the B-slab write-back: DMA drops 9.4 -> 7.3 MB/step, below the
# ~220 GB/s fabric's capacity within a step, so the PE stops stalling
# on the write-back chain.
B_SHIFTS = [(0, 1), (0, -1)]
# pass list per gate: (which tile, row offset)
PASSES = [("A", -1), ("A", 0), ("A", 1), ("B", -1), ("B", 0), ("B", 1)]


def build_kernel(nc, tc, x_ap, w_ap, b_ap, y_ap, n_steps):
    ctx_pools = []

    def pool(**kw):
        p = tc.tile_pool(**kw)
        ctx_pools.append(p)
        return p.__enter__()

    const = pool(name="const", bufs=1)
    tmp = pool(name="tmp", bufs=2)
    psum = pool(name="psum", bufs=8, space="PSUM")

    # Persistent tiles
    A = [const.tile([128, PL], BF16, tag=f"actA{i}", name=f"actA{i}") for i in range(2)]
    Bt = [const.tile([128, PL], BF16, tag=f"actB{i}", name=f"actB{i}") for i in range(2)]
    ctile = const.tile([128, GROUPS * NT], F32, tag="c")
    wsb = const.tile([128, 24 * 32], BF16, tag="w")
    bsb = const.tile([128, 4], F32, tag="bias")

    # head loads spread across rings so they overlap
    nc.scalar.dma_start(wsb[:, :], w_ap)
    nc.scalar.dma_start(bsb[:, :], b_ap)

    # one-time zero init.  B0 and A0's h block are fully read at t=0 and
    # must be all-zero; later steps only need the pad rows (never written by
    # the per-step h write-back) to be zero.  Chunked 34-row-wise so the
    # rows pair-0's first matmuls read are zeroed (and copied into A0)
    # first - cuts the t=0 head stall.
    # zero chunk 0 of B0 once, then source ALL A0-zero chunks from it so the
    # A0 fills don't serialize behind the remaining B0 memsets
    nc.vector.memset(Bt[0][:, 0 : 33 * W], 0.0)
    for ch in range(4):
        c0, c1 = ch * 33 * W, min((ch + 1) * 33 * W, PL)
        nc.gpsimd.dma_start(A[0][96:128, c0:c1], Bt[0][0:32, 0 : c1 - c0])
    for ch in range(1, 4):
        c0, c1 = ch * 33 * W, min((ch + 1) * 33 * W, PL)
        nc.vector.memset(Bt[0][:, c0:c1], 0.0)
    nc.vector.memset(ctile[:, :], 0.0)
    nc.vector.memset(Bt[1][:, 128 * W :], 0.0)          # pad rows 128..131
    nc.vector.memset(Bt[1][:, 0:W], 0.0)                # pad row 0
    nc.vector.memset(A[1][96:128, 0:W], 0.0)            # pad row 0
    nc.vector.memset(A[1][96:128, 129 * W :], 0.0)      # pad rows 129..131
    # b2/b3 partitions are never written in v7 but still streamed (x0
    # weights); they must hold zeros, not junk/NaN bit patterns
    nc.gpsimd.memset(Bt[1][64:128, 0 : 66 * W], 0.0)
    nc.gpsimd.memset(Bt[1][64:128, 66 * W :], 0.0)

    XCH = PL // 4  # 4224 elems -> 8.4KB descriptors, fine-grained ring interleave

    def load_x(t, chunk=None):
        # host provides 3 pre-shifted padded planes packed on partitions 0-95.
        # Chunked along the free dim so no descriptor exceeds ~8KB: SDMA
        # engines switch logical queues at packet granularity, and 33KB
        # descriptors block the latency-critical h write-backs for ~3us each.
        # Rides the sync/HWDGE ring: with the merged center writes (16
        # issues/step) the sync sequencer has slack, and keeping x off the
        # SWDGE ring keeps the slab-copy latency chain clean.
        chunks = range(4) if chunk is None else [chunk]
        for ch in chunks:
            nc.sync.dma_start(
                A[t % 2][0:96, XCH * ch : XCH * (ch + 1)],
                x_ap[t, :, XCH * ch : XCH * (ch + 1)],
            )

    load_x(0)

    deferred = []
    for t in range(n_steps):
        A_cur, B_cur = A[t % 2], Bt[t % 2]
        A_nxt, B_nxt = A[(t + 1) % 2], Bt[(t + 1) % 2]

        def pointwise(grp, zb, hbf, half):
            csl = ctile[:, grp * NT : (grp + 1) * NT]
            si = tmp.tile([128, NT], F32, tag="si", name=f"si{t}_{grp}")
            sf = tmp.tile([128, NT], F32, tag="sf", name=f"sf{t}_{grp}")
            so = tmp.tile([128, NT], F32, tag="so", name=f"so{t}_{grp}")
            tg = tmp.tile([128, NT], F32, tag="tg", name=f"tg{t}_{grp}")
            nc.scalar.activation(si[:, :], zb[0][:, :], AF.Sigmoid, bias=bsb[:, 0:1])
            nc.scalar.activation(sf[:, :], zb[1][:, :], AF.Sigmoid, bias=bsb[:, 1:2])
            nc.scalar.activation(so[:, :], zb[2][:, :], AF.Sigmoid, bias=bsb[:, 2:3])
            nc.scalar.activation(tg[:, :], zb[3][:, :], AF.Tanh, bias=bsb[:, 3:4])

            t2 = tmp.tile([128, NT], F32, tag="t2", name=f"t2_{t}_{grp}")
            t3 = tmp.tile([128, NT], F32, tag="t3", name=f"t3_{t}_{grp}")
            # t3 only needs sf (2nd activation) - run before t2 so the DVE
            # isn't blocked behind tanh_g on the critical chain
            nc.vector.tensor_mul(t3[:, :], sf[:, :], csl)
            nc.vector.tensor_mul(t2[:, :], si[:, :], tg[:, :])
            nc.vector.tensor_add(csl, t2[:, :], t3[:, :])

            tcn = tmp.tile([128, NT], F32, tag="tcn", name=f"tcn{t}_{grp}")
            nc.scalar.activation(tcn[:, :], csl, AF.Tanh)
            nc.vector.tensor_mul(
                hbf[:, half * NT : (half + 1) * NT], so[:, :], tcn[:, :]
            )

        def emit_centers(pair, hbf, A_nxt):
            # h -> A-center plane.  Both groups of the pair are merged into
            # one DMA per strip j ([32p, 2 segs of 1KB at 16-row stride],
            # 3-dim APs): 16 sync issues/step instead of 32 - the
            # ~0.65us/issue HWDGE sequencer cost was the v2 bottleneck
            # (sync engine 76% occupied).
            for j in range(TPG):
                d0 = (32 * pair + 4 * j + 1) * W
                v = A_nxt[96:128, d0 : d0 + NT]
                dst = BassAP(v.tensor, v.offset, [list(v.ap[0]), [16 * W, 2], [1, NT]])
                src = hbf[32 * j : 32 * j + 32, :].rearrange(
                    "p (g n) -> p g n", g=2
                )
                nc.sync.dma_start(dst, src)

        deferred_prev = deferred[:]
        del deferred[:]

        for pair in range(4):
            grps = (2 * pair, 2 * pair + 1)
            zbs = [
                [
                    psum.tile([128, NT], F32, tag="z", name=f"z{t}_{grp}_{g}")
                    for g in range(4)
                ]
                for grp in grps
            ]
            for g in range(4):
                for pi, (tsel, ry) in enumerate(PASSES):
                    src = A_cur if tsel == "A" else B_cur
                    col = (g * 6 + pi) * 32
                    lhsT = wsb[:, col : col + 32]
                    for j in range(TPG):
                        for gi, grp in enumerate(grps):
                            r0 = (16 * grp + 4 * j + 1 + ry) * W
                            nc.tensor.matmul(
                                zbs[gi][g][32 * j : 32 * j + 32, :],
                                lhsT,
                                src[:, r0 : r0 + NT],
                                start=(pi == 0),
                                stop=(pi == 5),
                                skip_group_check=True,
                                tile_position=(0, 32 * j),
                            )
            if pair == 1:
                # deferred end-of-previous-step emissions: after PAIR-1's
                # matmul emissions so neither pair-0 nor pair-1 matmuls
                # sem-lane-sweep them.  Legal: the deferred writes touch
                # A/B rows >= 96 and x chunk 3 (plane rows 99+), while
                # pair-0/1 matmuls read rows <= 66; pair-2 (rows 64..98)
                # is the first true reader of rows 96-98.
                for fn in deferred_prev:
                    fn()
                del deferred_prev[:]

            hbf = tmp.tile([128, 2 * NT], BF16, tag="hbf", name=f"hbf{t}_{pair}")
            if t + 1 < n_steps:
                pointwise(grps[0], zbs[0], hbf, 0)
                pointwise(grps[1], zbs[1], hbf, 1)
                if pair == 3:
                    # defer the last pair's centers into the next step so its
                    # first matmuls don't sweep them up in the sync sem-lane
                    # wait
                    deferred.append(
                        lambda pair=pair, hbf=hbf, A_nxt=A_nxt: emit_centers(
                            pair, hbf, A_nxt
                        )
                    )
                else:
                    emit_centers(pair, hbf, A_nxt)
            else:
                # last step: y straight from hbf (2-seg merged DMAs, no
                # A-center hop) - nothing else reads h after the final step
                pointwise(grps[0], zbs[0], hbf, 0)
                pointwise(grps[1], zbs[1], hbf, 1)
                for j in range(TPG):
                    d0 = (32 * pair + 4 * j) * W
                    v = y_ap[t][:, d0 : d0 + NT]
                    dst = BassAP(
                        v.tensor, v.offset, [list(v.ap[0]), [16 * W, 2], [1, NT]]
                    )
                    src = hbf[32 * j : 32 * j + 32, :].rearrange(
                        "p (g n) -> p g n", g=2
                    )
                    nc.sync.dma_start(dst, src)

            # shifted h copies into B_nxt for this 32-row slab + wrap-col
            # cleanup.  These ride the gpsimd/SWDGE path: a DMA issue that
            # must *wait* on a completion sem (slab after center) blocks its
            # sequencer, and gpsimd has nothing latency-critical behind it.
            lo = 32 * pair

            def emit_writeback(lo=lo, t=t, A_nxt=A_nxt, B_nxt=B_nxt, pair=pair):
                if t + 1 < n_steps:
                    for b_, (sy, sx) in enumerate(B_SHIFTS):
                        r_lo, r_hi = lo + 1 + sy, lo + 32 + sy  # dst rows
                        d0, d1 = r_lo * W, (r_hi + 1) * W
                        delta = -(sy * W + sx)
                        # trim one wrap-col element at the end (sx=-1) or
                        # start (sx=+1): src read stays within the pair rows
                        if sx == -1:
                            dd0, dd1 = d0, d1 - 1
                        else:
                            dd0, dd1 = d0 + 1, d1
                        nc.gpsimd.dma_start(
                            B_nxt[32 * b_ : 32 * b_ + 32, dd0:dd1],
                            A_nxt[96:128, dd0 + delta : dd1 + delta],
                        )
                    for b_, (sy, sx) in enumerate(B_SHIFTS):
                        r_lo, r_hi = lo + 1 + sy, lo + 32 + sy
                        wc = 0 if sx == 1 else W - 1
                        br = B_nxt[32 * b_ : 32 * b_ + 32, :].rearrange(
                            "p (r w) -> p r w", w=W
                        )
                        nc.gpsimd.memset(br[:, r_lo : r_hi + 1, wc : wc + 1], 0.0)
                    load_x(t + 1, chunk=pair)

            if pair == 3:
                # defer the end-of-step write-back emission into the next
                # step (after its pair-0 matmuls): nothing in the next step's
                # first pair reads these ranges, so emitting them later keeps
                # the pair-0 matmuls from waiting on this tail work.
                deferred.append(emit_writeback)
            else:
                emit_writeback()

        if t + 1 < n_steps:
            def emit_y(t=t, A_nxt=A_nxt):
                nc.sync.dma_start(y_ap[t], A_nxt[96:128, W : W + NSP])

            deferred.append(emit_y)

    for fn in deferred:
        fn()
    del deferred[:]

    for p in reversed(ctx_pools):
        p.__exit__(None, None, None)


def build_program(n_steps=S):
    nc = bacc.Bacc(
        "TRN2",
        target_bir_lowering=False,
        debug=False,
        enable_asserts=False,
        num_devices=N_CORES,
    )
    x_d = nc.dram_tensor("x", [n_steps, 96, PL], BF16, kind="ExternalInput")
    w_d = nc.dram_tensor("w", [128, 24 * 32], BF16, kind="ExternalInput")
    b_d = nc.dram_tensor("bias", [128, 4], F32, kind="ExternalInput")
    y_d = nc.dram_tensor("y", [n_steps, CO, NSP], BF16, kind="ExternalOutput")
    with tile.TileContext(nc) as tc:
        build_kernel(nc, tc, x_d.ap(), w_d.ap(), b_d.ap(), y_d.ap(), n_steps)
    nc.compile()
    return nc


def pack_weights(Wd):
    """Wd [128, 64, 3, 3] f32 -> lhsT blocks [128, 24*32] bf16.

    Block b = gate*6 + pass. K-row blocks of 32 channels; col m = out ch.
    A passes (0-2, ry=p-1): [x tap (ry,-1) | x (ry,0) | x (ry,+1) | h (ry,0)]
    B passes (3-5, ry=p-4): h taps [(ry,-1) | (ry,+1) | 0 | 0]
    """
    wp = np.zeros((128, 24, 32), np.float32)
    for g in range(4):
        Wg = Wd[g * 32 : (g + 1) * 32]  # [32(m), 64, 3, 3]
        for p in range(3):
            blk = wp[:, g * 6 + p, :]
            blk[0:32, :] = Wg[:, 0:32, p, 0].T
            blk[32:64, :] = Wg[:, 0:32, p, 1].T
            blk[64:96, :] = Wg[:, 0:32, p, 2].T
            blk[96:128, :] = Wg[:, 32:64, p, 1].T
        for p in range(3):
            blk = wp[:, g * 6 + 3 + p, :]
            blk[0:32, :] = Wg[:, 32:64, p, 0].T
            blk[32:64, :] = Wg[:, 32:64, p, 2].T
    return wp.reshape(128, 24 * 32).astype(ml_dtypes.bfloat16) if not globals().get("_NO_QUANT") else wp.reshape(128, 24 * 32)


def pack_bias(bd):
    """bd [128] f32 -> [128, 4] f32 (partition p = 32*tile + ch)."""
    bp = np.zeros((128, 4), np.float32)
    for g in range(4):
        bp[:, g] = np.tile(bd[g * 32 : (g + 1) * 32], 4)
    return bp


def pack_x(xc):
    """xc [S, 32, 128, 128] f32 -> [S, 96, NR*W] bf16: 3 shifted planes.

    partitions 0-31: x>>1 (plane(p,c)=x(p-1,c-1)), 32-63: center,
    64-95: x<<1 (plane(p,c)=x(p-1,c+1)). Host-exact zero padding.
    """
    n = xc.shape[0]
    xp = np.zeros((n, 96, NR, W), np.float32)
    xp[:, 32:64, 1 : H + 1, :] = xc
    xp[:, 0:32, 1 : H + 1, 1:] = xc[:, :, :, : W - 1]
    xp[:, 64:96, 1 : H + 1, : W - 1] = xc[:, :, :, 1:]
    return xp.reshape(n, 96, PL).astype(ml_dtypes.bfloat16) if not globals().get("_NO_QUANT") else xp.reshape(n, 96, PL)


def make_in_maps(x, Wf, bf, Wb, bb, n_steps=S):
    wpacks = [pack_weights(np.asarray(Wf, np.float32)),
              pack_weights(np.asarray(Wb, np.float32))]
    bpacks = [pack_bias(np.asarray(bf, np.float32)),
              pack_bias(np.asarray(bb, np.float32))]
    x = np.asarray(x, np.float32)
    in_maps = []
    for k in range(N_CORES):
        d, b = k // 4, k % 4
        xc = x[:n_steps, b] if d == 0 else x[::-1][:n_steps, b]
        in_maps.append(
            {
                "x": pack_x(xc),
                "w": wpacks[d],
                "bias": bpacks[d],
            }
        )
    return in_maps


_CACHED_NC = None


def kernel(x, Wf, bf, Wb, bb):
    global _CACHED_NC
    if _CACHED_NC is None:
        _CACHED_NC = build_program(S)
    nc = _CACHED_NC
    in_maps = make_in_maps(x, Wf, bf, Wb, bb)
    res = run_bass_kernel_spmd(nc, in_maps, core_ids=list(range(N_CORES)))
    out = np.empty((S, B, 2 * CO, H, W), np.float32)
    for k in range(N_CORES):
        d, b = k // 4, k % 4
        yk = res.results[k]["y"].astype(np.float32).reshape(S, CO, H, W)
        out[:, b, d * CO : (d + 1) * CO] = yk
    return out


if __name__ == "__main__":
    import jax

    jax.config.update("jax_platforms", "cpu")
    rng = np.random.default_rng(0)
    x = rng.standard_normal((S, B, CIN, H, W), np.float32)
    Wf = (rng.standard_normal((128, 64, 3, 3)) * 0.05).astype(np.float32)
    Wb = (rng.standard_normal((128, 64, 3, 3)) * 0.05).astype(np.float32)
    bf = np.zeros(128, np.float32)
    bb = np.zeros(128, np.float32)
    y = kernel(x, Wf, bf, Wb, bb)
    print("out", y.shape, y.dtype)

